# revision 7
# baseline (speedup 1.0000x reference)
"""Top-k threshold masking kernel for Trainium2 (Bass/Tile).

Computes, per row of x [2048, 32768] f32:
    threshold = (k+1)-th largest value of the row
    out = where(x >= threshold, x * 10, x)

Sharding: pure data-parallel over the batch (row) dim across 8 NeuronCores
(256 rows/core). Each core:
  - streams its [256, 32768] shard through SBUF in [128, 32768] row-tiles,
  - per free-dim chunk, computes the chunk top-8 with the DVE max8 op while
    the DMA load streams in,
  - merges chunk top-8s -> row top-8 -> threshold = top8[:, k],
  - second pass over resident SBUF data: mask = (x >= thr) on DVE,
    x10 = 10*x on the scalar engine, patch x in place via copy_predicated,
    and DMA the patched tile out.
Data is read from HBM exactly once and written exactly once (memory-bound
regime; ~64 MiB of HBM traffic per core).
"""

import numpy as np

import concourse.bacc as bacc
import concourse.bass as bass
import concourse.mybir as mybir
from concourse.bass_utils import run_bass_kernel_spmd
from concourse.tile import TileContext

N_CORES = 8
B, N = 2048, 32768
ROWS_PER_CORE = B // N_CORES  # 256
P = 128
TILES_PER_CORE = ROWS_PER_CORE // P  # 2
CHUNK = 2048
N_CHUNKS = N // CHUNK

_nc_cache: dict[int, bass.Bass] = {}


def _build(k: int) -> bass.Bass:
    assert 0 <= k <= 7, f"k={k} needs top-(k+1) which must fit in max8's top-8"
    nc = bacc.Bacc("TRN2", target_bir_lowering=False)
    x = nc.dram_tensor("x", [ROWS_PER_CORE, N], mybir.dt.float32, kind="ExternalInput")
    out = nc.dram_tensor(
        "out", [ROWS_PER_CORE, N], mybir.dt.float32, kind="ExternalOutput"
    )

    with TileContext(nc) as tc:
        with (
            tc.tile_pool(name="big", bufs=1) as big_pool,
            tc.tile_pool(name="chunks", bufs=2) as chunk_pool,
            tc.tile_pool(name="small", bufs=2) as small_pool,
        ):
            for t in range(TILES_PER_CORE):
                rows = slice(t * P, (t + 1) * P)
                xt = big_pool.tile([P, N], mybir.dt.float32, tag="xt")
                cand = small_pool.tile([P, 8 * N_CHUNKS], mybir.dt.float32, tag="cand")
                for c in range(N_CHUNKS):
                    sl = slice(c * CHUNK, (c + 1) * CHUNK)
                    nc.sync.dma_start(out=xt[:, sl], in_=x[rows, sl])
                    nc.vector.max(out=cand[:, c * 8 : (c + 1) * 8], in_=xt[:, sl])
                top8 = small_pool.tile([P, 8], mybir.dt.float32, tag="top8")
                nc.vector.max(out=top8, in_=cand)
                thr = top8[:, k : k + 1]
                for c in range(N_CHUNKS):
                    sl = slice(c * CHUNK, (c + 1) * CHUNK)
                    x10 = chunk_pool.tile([P, CHUNK], mybir.dt.float32, tag="x10")
                    mask = chunk_pool.tile([P, CHUNK], mybir.dt.uint8, tag="mask")
                    nc.vector.tensor_scalar_mul(x10, xt[:, sl], 10.0)
                    nc.vector.tensor_scalar(
                        mask, xt[:, sl], thr, None, op0=mybir.AluOpType.is_lt
                    )
                    nc.vector.copy_predicated(x10, mask, xt[:, sl])
                    nc.scalar.dma_start(out=out[rows, sl], in_=x10)
    nc.compile()
    return nc


def kernel(x: np.ndarray, k) -> np.ndarray:
    k = int(k)
    if k not in _nc_cache:
        _nc_cache[k] = _build(k)
    nc = _nc_cache[k]

    x = np.ascontiguousarray(x, dtype=np.float32)
    in_maps = [
        {"x": x[i * ROWS_PER_CORE : (i + 1) * ROWS_PER_CORE]} for i in range(N_CORES)
    ]
    res = run_bass_kernel_spmd(nc, in_maps, core_ids=list(range(N_CORES)))
    return np.concatenate([r["out"] for r in res.results], axis=0)
